# revision 1
# baseline (speedup 1.0000x reference)
"""Trainium2 Bass kernel for nn_MultiLIF_17059610100026.

Adaptive LIF neuron layer: for input I[B=32, L=1024, K=512], runs the
per-(b,k) time recurrence

    th     = 1.5 + 1.5*a
    v_pre  = 0.95*v + I_t          (scan-form rounding)
    s      = (v_pre >= th)
    sn    += s
    v      = s ? -0.5 : v_pre
    a      = 0.99*a + s

and returns (spikes, series, v_seq), each [B, L, K] f32.

Sharding: fully data-parallel over B — core c gets b in [4c, 4c+4).
Per-core layout: partition p = k % 128, free neuron n = 4*b_loc + k//128
(16 per partition), time in blocks of T; SBUF tiles [128, 16*T] with
per-neuron time-contiguous segments.

Per time step the whole per-core state is one [128, 16] tile; the serial
recurrence runs as 6 fused DVE ops/step. The `series` output is produced
per block with a single segmented tensor_tensor_scan, and the u8 spike
mask is cast to f32 on the scalar engine.
"""
import numpy as np

B, L, K = 32, 1024, 512
NCORES = 8
B_LOC = B // NCORES          # 4
P = 128                      # partitions
KH = K // P                  # 4 k-groups
NN = B_LOC * KH              # 16 neurons per partition
T = 64                       # time block
NBLK = L // T

_cache = {}


def _legalize_waits(nc, max_waits=1):
    """Split multi-wait instructions into chains of single-wait NoOps.

    The walrus build here rejects instructions carrying more than one
    sync-wait. Hoist extra waits onto NoOps on the same engine right
    before the instruction (engines execute in order, so this is
    semantically identical).
    """
    import concourse.mybir as mybir

    n = 0
    ctr = [0]
    for fn in nc.m.functions:
        for blk in fn.blocks:
            insts = list(blk.instructions)
            out = []
            changed = False
            for ins in insts:
                si = ins.sync_info
                waits = list(si.on_wait) if (si is not None and si.on_wait) else []
                if len(waits) > max_waits:
                    for w in waits[max_waits:]:
                        ctr[0] += 1
                        nop = mybir.InstNoOp(name=f"legal-wait-nop-{ctr[0]}")
                        nop.engine = ins.engine
                        nop.sync_info = mybir.SyncInfo(on_wait=[w], on_update=[])
                        out.append(nop)
                    ins.sync_info = mybir.SyncInfo(
                        on_wait=waits[:max_waits],
                        on_update=list(si.on_update or []),
                    )
                    changed = True
                    n += 1
                out.append(ins)
            if changed:
                blk.instructions = out
    return n


def _build(nblk=NBLK):
    import concourse.bass as bass
    import concourse.mybir as mybir
    from concourse.tile import TileContext

    f32 = mybir.dt.float32
    u8 = mybir.dt.uint8
    A = mybir.AluOpType

    nc = bass.Bass()
    I_d = nc.declare_dram_parameter("I", [B_LOC, L, K], f32, isOutput=False)
    spk_d = nc.declare_dram_parameter("spikes", [B_LOC, L, K], f32, isOutput=True)
    ser_d = nc.declare_dram_parameter("series", [B_LOC, L, K], f32, isOutput=True)
    vsq_d = nc.declare_dram_parameter("v_seq", [B_LOC, L, K], f32, isOutput=True)

    def dram_view(t, blk):
        # per-(b,kh) 2D DMA views of DRAM block slice; SBUF side [128, T]
        views = []
        for b in range(B_LOC):
            for kh in range(KH):
                views.append(
                    t[b, blk * T:(blk + 1) * T, kh * P:(kh + 1) * P]
                    .rearrange("l p -> p l")
                )
        return views

    with TileContext(nc) as tc:
        with (
            tc.tile_pool(name="state", bufs=1) as stp,
            tc.tile_pool(name="io", bufs=2) as iop,
        ):
            v_post = stp.tile([P, NN], f32, tag="v_post")
            a = stp.tile([P, NN], f32, tag="a")
            sn_carry = stp.tile([P, NN], f32, tag="sn_carry")
            neghalf = stp.tile([P, NN], f32, tag="neghalf")
            th = stp.tile([P, NN], f32, tag="th")
            d0sn = stp.tile([P, NN * T], f32, tag="d0sn")

            nc.vector.memset(v_post[:], 0.0)
            nc.vector.memset(a[:], 0.0)
            nc.vector.memset(sn_carry[:], 0.0)
            nc.vector.memset(neghalf[:], -0.5)
            nc.vector.memset(d0sn[:], 1.0)
            d0v = d0sn[:].rearrange("p (n t) -> p n t", t=T)
            nc.vector.memset(d0v[:, :, 0:1], 0.0)

            for blk in range(nblk):
                Xi = iop.tile([P, NN * T], f32, tag="Xi")
                Vst = iop.tile([P, NN * T], f32, tag="Vst")
                S8 = iop.tile([P, NN * T], u8, tag="S8")
                Sf = iop.tile([P, NN * T], f32, tag="Sf")
                SN = iop.tile([P, NN * T], f32, tag="SN")

                for n, v in enumerate(dram_view(I_d, blk)):
                    nc.sync.dma_start(out=Xi[:, n * T:(n + 1) * T], in_=v)

                Xv = Xi[:].rearrange("p (n t) -> p n t", t=T)
                Vv = Vst[:].rearrange("p (n t) -> p n t", t=T)
                S8v = S8[:].rearrange("p (n t) -> p n t", t=T)

                for tau in range(T):
                    nc.vector.scalar_tensor_tensor(
                        out=Vv[:, :, tau], in0=v_post[:], scalar=0.95,
                        in1=Xv[:, :, tau], op0=A.mult, op1=A.add)
                    nc.vector.tensor_scalar(
                        out=th[:], in0=a[:], scalar1=1.5, scalar2=1.5,
                        op0=A.mult, op1=A.add)
                    nc.vector.tensor_tensor(
                        out=S8v[:, :, tau], in0=Vv[:, :, tau], in1=th[:],
                        op=A.is_ge)
                    nc.vector.tensor_copy(out=v_post[:], in_=Vv[:, :, tau])
                    nc.vector.copy_predicated(
                        out=v_post[:], mask=S8v[:, :, tau], data=neghalf[:])
                    nc.vector.scalar_tensor_tensor(
                        out=a[:], in0=a[:], scalar=0.99,
                        in1=S8v[:, :, tau], op0=A.mult, op1=A.add)

                # v_seq out
                for n, v in enumerate(dram_view(vsq_d, blk)):
                    nc.sync.dma_start(out=v, in_=Vst[:, n * T:(n + 1) * T])
                # spikes: cast u8 -> f32 on ACT, then out
                nc.scalar.copy(out=Sf[:], in_=S8[:])
                for n, v in enumerate(dram_view(spk_d, blk)):
                    nc.sync.dma_start(out=v, in_=Sf[:, n * T:(n + 1) * T])
                # series: add carry into col 0, segmented prefix-sum scan
                Sfv = Sf[:].rearrange("p (n t) -> p n t", t=T)
                nc.vector.tensor_tensor(
                    out=Sfv[:, :, 0], in0=Sfv[:, :, 0], in1=sn_carry[:],
                    op=A.add)
                nc.vector.tensor_tensor_scan(
                    out=SN[:], data0=d0sn[:], data1=Sf[:], initial=0.0,
                    op0=A.mult, op1=A.add)
                SNv = SN[:].rearrange("p (n t) -> p n t", t=T)
                nc.scalar.copy(out=sn_carry[:], in_=SNv[:, :, T - 1])
                for n, v in enumerate(dram_view(ser_d, blk)):
                    nc.sync.dma_start(out=v, in_=SN[:, n * T:(n + 1) * T])

    _legalize_waits(nc)
    return nc


def kernel(I, _trace=False, _nblk=NBLK):
    from concourse.bass_utils import run_bass_kernel_spmd

    I = np.ascontiguousarray(np.asarray(I, dtype=np.float32))
    key = _nblk
    if key not in _cache:
        _cache[key] = _build(_nblk)
    nc = _cache[key]

    in_maps = [{"I": I[c * B_LOC:(c + 1) * B_LOC]} for c in range(NCORES)]
    out = run_bass_kernel_spmd(nc, in_maps, list(range(NCORES)), trace=_trace)
    res = out.results
    spikes = np.concatenate([res[c]["spikes"] for c in range(NCORES)], axis=0)
    series = np.concatenate([res[c]["series"] for c in range(NCORES)], axis=0)
    v_seq = np.concatenate([res[c]["v_seq"] for c in range(NCORES)], axis=0)
    if _trace:
        kernel._last = out
    return spikes, series, v_seq
